# revision 73
# baseline (speedup 1.0000x reference)
"""Distributed kNN OOD-score kernel for 8 Trainium2 NeuronCores.

Problem: for each of 4*32*32 query vectors (D=768), find the 3 nearest
database vectors (N=20000, squared-L2), average the 3 distances, and
bilinearly upsample the resulting [4,32,32] map to [4,1,512,512].

Sharding: queries are data-parallel. Each core owns half of one batch
image (16 of 32 query rows = 512 queries); the database is replicated
and streamed through SBUF in fp8 (e4m3). The one halo row each core
needs for the 16x bilinear upsample is exchanged with its pair core via
a tiny AllGather whose ~15us fixed latency is hidden by hoisting the
boundary query tile (qt0) of the last two super-chunks ahead of the
other tiles (the fp8 database tiles stay resident long enough to allow
the out-of-order walk).

Per-core device program, per (super-chunk sc, query-tile qt):
  - scores t[q,n] = q.x - (||x||^2 - 768)/2 via TensorE fp8 DoubleRow
    matmuls: 3 chunks of K=256 at 0.5 cycles/row (4x bf16 rate) + one
    K=4 DoubleRow adding the centered -||x||^2/2 row in 4-way split-fp8
    precision, accumulated f32 in PSUM (8 banks, bank-major fill).
  - ScalarE evacuates PSUM banks 0-5 (two [128,3,500] copies) into a
    bf16 strip; VectorE max-fuses banks 6-7 against strip[0:1000]
    (TensorTensor may read one PSUM operand), then a bf16 pairwise-max
    ladder (2x DVE mode) reduces 4000 -> 250 candidates; per sc-pair one
    max8 keeps the top-8 (exact top-3 up to ~0.1%/query collisions in
    the depth-16 max groups, well inside tolerance).
  - final max8 over 24 strip winners -> top-3 scores; mean distance =
    reduce_sum of sqrt((q^2+768-2t)/9) (ScalarE fused scale+bias+sqrt).
  - 16x bilinear upsample = two small bf16 matmuls with interpolation
    matrices (host-built; weights are dyadic so bf16-exact).
"""

import sys

if "/opt/trn_rl_repo" not in sys.path:
    sys.path.insert(0, "/opt/trn_rl_repo")

import numpy as np
import ml_dtypes

import concourse.bass as bass
import concourse.bacc as bacc
import concourse.mybir as mybir
import concourse.tile as tile
from concourse import bass_utils

# Problem shape (hardcoded per contract).
B, D, H, W = 4, 768, 32, 32
N = 20000
K_NN = 3
OUT_H = OUT_W = 512
N_CORES = 8

SC = 4000            # db columns per super-chunk
N_SC = N // SC       # 5
BANK = 500           # db columns per PSUM bank
CH = D // 256        # 3 DoubleRow contraction chunks (K=256 each)
QPC = 512            # queries scored per core (16 rows)
N_QT = QPC // 128    # 4
OROWS = 256          # output rows per core
NCOL = 24            # ood columns entering the upsample (16 own + 2x4 gathered)

F32 = mybir.dt.float32
BF16 = mybir.dt.bfloat16
FP8 = mybir.dt.float8e4
AX = mybir.AxisListType
AF = mybir.ActivationFunctionType
DR = mybir.MatmulPerfMode.DoubleRow

# local tile -> 4-row block of this core's half (block i = rows 4i..4i+3).
# Tile 0 is the block the PAIR core needs as its halo row: for the top
# half (rows 0-15) that's block 3 (row 15), for the bottom half (rows
# 16-31) block 0 (row 16).
TILE_BLOCKS = ([3, 0, 1, 2], [0, 1, 2, 3])

# qt0-2 of the last two super-chunks are hoisted so qt0's ood (the
# boundary block) finishes ~15us before the instruction stream ends
# (hiding the AllGather's fixed latency) and only qt3's ladder + final
# remain as post-matmul tail work.
PHASES = [
    (0, (0, 1, 2, 3)),
    (1, (0, 1, 2, 3)),
    (3, (0,)),
    (4, (0,)),
    (3, (1, 2)),
    (2, (0, 1, 2, 3)),
    (4, (1, 2)),
    (3, (3,)),
    (4, (3,)),
]


def _build_program():
    nc = bacc.Bacc(
        "TRN2", target_bir_lowering=False, debug=False, num_devices=N_CORES
    )
    dbd = nc.dram_tensor("dbd", [128, N_SC, CH, 2, SC], FP8, kind="ExternalInput").ap()
    qd = nc.dram_tensor("qd", [128, N_QT, CH, 2, 128], FP8, kind="ExternalInput").ap()
    xhd = nc.dram_tensor("xhd", [2, 2, N], FP8, kind="ExternalInput").ap()
    q2 = nc.dram_tensor("q2", [128, N_QT], F32, kind="ExternalInput").ap()
    art = nc.dram_tensor("art", [40, OROWS], BF16, kind="ExternalInput").ap()
    ac4 = nc.dram_tensor("ac4", [128, OUT_W], BF16, kind="ExternalInput").ap()
    msk4 = nc.dram_tensor("msk4", [128, 4], BF16, kind="ExternalInput").ap()
    out = nc.dram_tensor("out", [OROWS, OUT_W], F32, kind="ExternalOutput").ap()

    with tile.TileContext(nc) as tc:
        with (
            tc.tile_pool(name="static", bufs=1) as sp,
            tc.tile_pool(name="db", bufs=3) as dbp,
            tc.tile_pool(name="strip", bufs=2) as stp,
            tc.tile_pool(name="lad", bufs=2) as ltp,
            tc.tile_pool(name="small", bufs=2) as smp,
            tc.tile_pool(name="psum", bufs=1, space="PSUM") as pp,
            tc.tile_pool(name="dram", bufs=1, space="DRAM") as dp,
        ):
            # DMA issue order is the DMA-engine FIFO order: the first db
            # slice leads on SP while the queries/shift rows go through the
            # Activation-issued HWDGE queue in parallel.
            qk = sp.tile([128, N_QT, CH, 2, 128], FP8, name="qk")
            nc.sync.dma_start(qk[:], qd[:])
            xh4 = sp.tile([2, 2, N], FP8, name="xh4")
            nc.sync.dma_start(xh4[:], xhd[:])
            db_t = {}
            db_t[0] = dbp.tile([128, CH, 2, SC], FP8, tag="db", name="db0")
            for h in range(4):
                nc.sync.dma_start(
                    db_t[0][:, :, :, h * 1000 : (h + 1) * 1000],
                    dbd[:, 0, :, :, h * 1000 : (h + 1) * 1000],
                )
            q2_sb = sp.tile([128, N_QT], F32, name="q2_sb")
            nc.sync.dma_start(q2_sb[:], q2[:])
            art_sb = sp.tile([40, OROWS], BF16, name="art_sb")
            nc.sync.dma_start(art_sb[:], art[:])
            ac4_sb = sp.tile([128, OUT_W], BF16, name="ac4_sb")
            nc.sync.dma_start(ac4_sb[:], ac4[:])
            msk4_sb = sp.tile([128, 4], BF16, name="msk4_sb")
            nc.sync.dma_start(msk4_sb[:], msk4[:])
            ones4 = sp.tile([2, 2, 128], FP8, name="ones4")
            nc.gpsimd.memset(ones4[:], 1.0)
            warm = sp.tile([2, 2, 512], FP8, name="warm")
            nc.gpsimd.memset(warm[:], 0.0)

            # db tiles: 4-buffer rotation; DMA-FIFO order matches the phase
            # walk (sc0, sc1, sc3, sc4/sc2 interleaved) so the hoisted qt0
            # phases never starve.  sc2 reuses sc0's buffer.
            for sc in (1, 3):
                t = dbp.tile([128, CH, 2, SC], FP8, tag="db", name=f"db{sc}")
                db_t[sc] = t
                for h in range(2):
                    nc.sync.dma_start(
                        t[:, :, :, h * 2000 : (h + 1) * 2000],
                        dbd[:, sc, :, :, h * 2000 : (h + 1) * 2000],
                    )
            db_t[4] = dbp.tile([128, CH, 2, SC], FP8, tag="db", name="db4")
            db_t[2] = dbp.tile([128, CH, 2, SC], FP8, tag="db", name="db2")
            for sc, h in ((4, 0), (4, 1), (2, 0), (2, 1)):
                nc.sync.dma_start(
                    db_t[sc][:, :, :, h * 2000 : (h + 1) * 2000],
                    dbd[:, sc, :, :, h * 2000 : (h + 1) * 2000],
                )

            # PSUM: four 2-bank pair tiles; banks 0-5 (ScalarE evac) and
            # 6-7 (DVE fused max) in the standard per-scqt walk, one pair
            # per query tile in the DMA-paced phase 0.
            P = [
                pp.tile([128, 2, 512], F32, tag=f"p{i}", name=f"P{i}")
                for i in range(4)
            ]

            # p-state warmup: ~3us of throwaway PE work during the db DMA
            # wait so the first real matmuls run at full clock
            for _ in range(6):
                nc.tensor.matmul(
                    P[3][:, 0, :], ones4[:], warm[:], start=True, stop=True,
                    perf_mode=DR,
                )

            Dbuf = sp.tile([128, N_QT, 4, 250], BF16, name="Dbuf")
            # per-qt final candidates: [0:16] = max8 of sc-pairs (0,1) and
            # (2,3); [16:516] = sc4's ladder output at the 500-wide level
            # (skips two reduction hops on the critical tail)
            parts = sp.tile([128, N_QT, 516], BF16, name="parts")
            oods_all = sp.tile([128, N_QT], F32, name="oods_all")
            ood0_bf = sp.tile([128, 1], BF16, name="ood0_bf")
            # SEL[p, 4qt+a] = ood[p, qt] * [p//32 == a]; SELc holds the
            # gathered boundary blocks on partitions 0-31 (zero elsewhere).
            # With ac4[p, ow] = Ac.T[p%32, ow] the K=128 matmuls SEL.T @ ac4
            # implicitly transpose lanes.  Separate tiles so the AllGather's
            # DMA into SELc carries no false dep on the qt3 SEL write.
            SEL = sp.tile([128, 16], BF16, name="SEL")
            SELc = sp.tile([128, 8], BF16, name="SELc")
            nc.gpsimd.memset(SELc[:], 0.0)
            cc_in = dp.tile([128], BF16, name="cc_in")
            cc_out = dp.tile([256], BF16, name="cc_out")

            def emit_mms(pt, qt, sc, col):
                o = pt[:, (col // BANK) % 2, 0:BANK]
                for ch in range(CH):
                    nc.tensor.matmul(
                        o,
                        qk[:, qt, ch],
                        db_t[sc][:, ch, :, col : col + BANK],
                        start=(ch == 0),
                        stop=False,
                        perf_mode=DR,
                    )
                nc.tensor.matmul(
                    o,
                    ones4[:],
                    xh4[:, :, sc * SC + col : sc * SC + col + BANK],
                    start=False,
                    stop=True,
                    perf_mode=DR,
                )

            def emit_scqt(sc, qt):
                # bank-major fill so PSUM groups complete in evac order
                for b in range(8):
                    emit_mms(P[b // 2], qt, sc, BANK * b)
                strip = stp.tile([128, 3000], BF16, tag="strip", name="strip")
                sv = strip.rearrange("p (a b n) -> p a b n", a=3, b=2)
                for i in range(3):
                    nc.scalar.activation(sv[:, i], P[i][:, :, 0:BANK], AF.Copy)
                # pairwise-max ladder: 4000 -> 250 exact-value candidates,
                # chained so each level consumes an evac slice as it lands
                Ht = ltp.tile([128, 1000], BF16, tag="H", name="Ht")
                nc.vector.tensor_max(Ht[:], P[3][:, :, 0:BANK], strip[:, 0:1000])
                At = ltp.tile([128, 1000], BF16, tag="A", name="At")
                nc.vector.tensor_max(At[:], strip[:, 1000:2000], Ht[:])
                Bt = ltp.tile([128, 1000], BF16, tag="Bv", name="Bt")
                nc.vector.tensor_max(Bt[:], strip[:, 2000:3000], At[:])
                if sc < 4:
                    Ct = ltp.tile([128, 500], BF16, tag="C", name="Ct")
                    nc.vector.tensor_max(Ct[:], Bt[:, 0:500], Bt[:, 500:1000])
                    nc.vector.tensor_max(
                        Dbuf[:, qt, sc], Ct[:, 0:250], Ct[:, 250:500]
                    )
                else:
                    nc.vector.tensor_max(
                        parts[:, qt, 16:516], Bt[:, 0:500], Bt[:, 500:1000]
                    )

            # Phase 0 is paced by the sc0 DMA stream: process all four
            # query tiles per arriving 1000-col quarter (one PSUM pair
            # each).  qt0-2 evacuate via ScalarE into per-qt strips; qt3's
            # quarters fold into a running pairwise max on DVE.
            ph0s = sp.tile([128, 3, SC], BF16, name="ph0s")
            h3 = [
                sp.tile([128, 1000], BF16, name=f"h3{i}") for i in range(2)
            ]
            ph0sv = ph0s.rearrange("p q (r b n) -> p q r b n", r=4, b=2)

            def emit_phase0():
                for r in range(4):
                    for qt in range(N_QT):
                        for b2 in range(2):
                            emit_mms(P[qt], qt, 0, 1000 * r + BANK * b2)
                    for qt in range(3):
                        nc.scalar.activation(
                            ph0sv[:, qt, r], P[qt][:, :, 0:BANK], AF.Copy
                        )
                    if r == 0:
                        nc.vector.tensor_copy(h3[0][:], P[3][:, :, 0:BANK])
                    else:
                        nc.vector.tensor_max(
                            h3[r % 2][:], P[3][:, :, 0:BANK], h3[1 - r % 2][:]
                        )
                for qt in range(3):
                    s = ph0s[:, qt]
                    A = ltp.tile([128, 2000], BF16, tag="H", name="A0")
                    nc.vector.tensor_max(A[:], s[:, 0:2000], s[:, 2000:4000])
                    Bt = ltp.tile([128, 1000], BF16, tag="A", name="B0")
                    nc.vector.tensor_max(Bt[:], A[:, 0:1000], A[:, 1000:2000])
                    Ct = ltp.tile([128, 500], BF16, tag="C", name="C0")
                    nc.vector.tensor_max(Ct[:], Bt[:, 0:500], Bt[:, 500:1000])
                    nc.vector.tensor_max(
                        Dbuf[:, qt, 0], Ct[:, 0:250], Ct[:, 250:500]
                    )
                C3 = ltp.tile([128, 500], BF16, tag="C", name="C3")
                nc.vector.tensor_max(C3[:], h3[1][:, 0:500], h3[1][:, 500:1000])
                nc.vector.tensor_max(Dbuf[:, 3, 0], C3[:, 0:250], C3[:, 250:500])

            def emit_pair_max8(qt, pair_idx):
                i0 = pair_idx * 8
                src = Dbuf[:, qt, 2 * pair_idx : 2 * pair_idx + 2]
                nc.vector.max(parts[:, qt, i0 : i0 + 8], src)

            def emit_qt_final(qt):
                f8 = smp.tile([128, 8], BF16, tag="f8", name="f8")
                nc.vector.max(f8[:], parts[:, qt])
                # dist_j/3 = sqrt((q2 + 768 - 2 t_j) / 9); host passes
                # (q2+768)/9 as the bias
                d3 = smp.tile([128, K_NN], BF16, tag="d3", name="d3")
                nc.scalar.activation(
                    d3[:],
                    f8[:, 0:K_NN],
                    AF.Sqrt,
                    bias=q2_sb[:, qt : qt + 1],
                    scale=-2.0 / 9.0,
                    accum_out=oods_all[:, qt : qt + 1],
                )
                nc.vector.tensor_scalar_mul(
                    SEL[:, 4 * qt : 4 * qt + 4],
                    msk4_sb[:],
                    oods_all[:, qt : qt + 1],
                )
                if qt == 0:
                    # boundary block: gather across the pair while qt1-3
                    # of sc3/sc4 still run (~25us of cover); issue from the
                    # otherwise-idle Pool queue so it never waits on SP
                    nc.vector.tensor_copy(ood0_bf[:], oods_all[:, 0:1])
                    nc.gpsimd.dma_start(cc_in[:], ood0_bf[:])
                    nc.gpsimd.collective_compute(
                        "AllGather",
                        mybir.AluOpType.bypass,
                        replica_groups=[[0, 1], [2, 3], [4, 5], [6, 7]],
                        ins=[cc_in.opt()],
                        outs=[cc_out.opt()],
                    )
                    nc.sync.dma_start(
                        SELc[0:32, :],
                        cc_out.rearrange("(b r c) -> c (b r)", b=2, c=W),
                    )

            # qt0 walks its super-chunks in order 0,1,3,4,2 so its final
            # (and the AllGather) launches ~25us before the stream ends;
            # qt1/qt2 walk 0,1,3,2,4 and qt3 walks in order, so the phase
            # sequence never outruns the db DMA stream.
            for sc, qts in PHASES:
                if sc == 0:
                    emit_phase0()
                    continue
                for qt in qts:
                    emit_scqt(sc, qt)
                    if sc == 1:
                        emit_pair_max8(qt, 0)
                    elif sc == 2 and qt == 0:
                        emit_pair_max8(0, 1)
                        emit_qt_final(0)
                    elif sc == 2 and qt in (1, 2):
                        emit_pair_max8(qt, 1)
                    elif sc == 3 and qt == 3:
                        emit_pair_max8(3, 1)
                    elif sc == 4 and qt != 0:
                        emit_qt_final(qt)

            # Upsample: out = art.T @ (SEL.T @ ac4); the K=128 first stage
            # transposes the ood lanes implicitly.  p1p rows 0:16 = own, 32:40
            # = gathered (PE column offset 32 is the only legal slot for an
            # M=8 output); rows 16:32 are stale-but-finite PSUM ignored via
            # zero rows of art.
            p1p = pp.tile([40, OUT_W], F32, tag="p0", name="p1p")
            nc.tensor.matmul(
                p1p[32:40, :], SELc[:], ac4_sb[:], start=True, stop=True,
                tile_position=(0, 32),
            )
            nc.tensor.matmul(p1p[0:16, :], SEL[:], ac4_sb[:], start=True, stop=True)
            p1_sb = sp.tile([40, OUT_W], BF16, name="p1_sb")
            nc.scalar.activation(p1_sb[:], p1p[:], AF.Copy)
            # parallel output ceremony: half 0 via Act copy + SP DMA,
            # half 1 via DVE copy + Act-issued DMA
            for m in range(2):
                p2p = pp.tile(
                    [128, OUT_W], F32, tag=("p1", "p2")[m], name=f"p2p{m}"
                )
                nc.tensor.matmul(
                    p2p[:],
                    art_sb[:, m * 128 : (m + 1) * 128],
                    p1_sb[:],
                    start=True,
                    stop=True,
                )
                o_sb = smp.tile([128, OUT_W], F32, tag="osb", name=f"osb{m}")
                if m == 0:
                    nc.scalar.activation(o_sb[:], p2p[:], AF.Copy)
                    nc.sync.dma_start(out[0:128, :], o_sb[:])
                else:
                    nc.vector.tensor_copy(o_sb[:], p2p[:])
                    nc.scalar.dma_start(out[128:256, :], o_sb[:])

    nc.compile()
    return nc


def _bilinear_matrix(out_size: int, in_size: int) -> np.ndarray:
    """Half-pixel (align_corners=False) bilinear interpolation matrix
    [out_size, in_size]; edge-clamped, equivalent to jax.image.resize
    'bilinear' for integer upsampling."""
    A = np.zeros((out_size, in_size), dtype=np.float64)
    scale = in_size / out_size
    for i in range(out_size):
        s = (i + 0.5) * scale - 0.5
        j0 = int(np.floor(s))
        w = s - j0
        A[i, min(max(j0, 0), in_size - 1)] += 1.0 - w
        A[i, min(max(j0 + 1, 0), in_size - 1)] += w
    return A.astype(np.float32)


_NC_CACHE = None


def _get_nc():
    global _NC_CACHE
    if _NC_CACHE is None:
        _NC_CACHE = _build_program()
    return _NC_CACHE


def _fp8_split4(v: np.ndarray) -> np.ndarray:
    """4-way residual split of f32 vector v into fp8 e4m3 rows that sum
    (in f32) back to v to ~1e-4 absolute."""
    rows = []
    r = v.astype(np.float64)
    for _ in range(4):
        s = np.asarray(r, dtype=np.float32).astype(ml_dtypes.float8_e4m3)
        rows.append(s)
        r = r - s.astype(np.float64)
    return np.stack(rows)  # [4, N]


def make_in_maps(embeddings: np.ndarray, database: np.ndarray):
    embeddings = np.asarray(embeddings, dtype=np.float32)
    database = np.asarray(database, dtype=np.float32)

    x8 = database.astype(ml_dtypes.float8_e4m3)          # [N, D]
    # dbd[p, sc, ch, i, n] = x8[sc*SC+n, 256ch+128i+p]
    dbT = np.ascontiguousarray(x8.T)                     # [D, N]
    dbd = np.ascontiguousarray(
        dbT.reshape(CH, 2, 128, N_SC, SC).transpose(2, 3, 0, 1, 4)
    )
    # centered score shift s = (768 - ||x8||^2)/2, 4-way split fp8
    xs = x8.astype(np.float32)
    s = (D - np.einsum("nd,nd->n", xs, xs)) * 0.5
    sp4 = _fp8_split4(s)                                 # [4, N]
    xhd = np.ascontiguousarray(sp4.reshape(2, 2, N).transpose(1, 0, 2))

    q_all = embeddings.transpose(0, 2, 3, 1).reshape(B, H * W, D)
    Ac = _bilinear_matrix(OUT_W, W)                      # [512, 32]
    Ar = _bilinear_matrix(OUT_H, H)                      # [512, 32]
    # the two gathered blocks in cc_out rank order: pair-core tile 0 rows
    cc_rows = [12, 13, 14, 15, 16, 17, 18, 19]

    in_maps = []
    for c in range(N_CORES):
        b, half = divmod(c, 2)
        blocks = TILE_BLOCKS[half]
        own_rows = [16 * half + 4 * blk + r for blk in blocks for r in range(4)]

        # queries in local-tile order
        q = np.concatenate(
            [
                q_all[b, (16 * half + 4 * blk) * W : (16 * half + 4 * blk + 4) * W]
                for blk in blocks
            ]
        )                                                # [512, 768]
        q8 = q.astype(ml_dtypes.float8_e4m3)
        # qd[p, qt, ch, i, m] = q8[128qt+m, 256ch+128i+p]
        qT = np.ascontiguousarray(q8.T)                  # [768, 512]
        qdc = np.ascontiguousarray(
            qT.reshape(CH, 2, 128, N_QT, 128).transpose(2, 3, 0, 1, 4)
        )
        q8f = q8.astype(np.float32)
        q2v = (np.einsum("qd,qd->q", q8f, q8f) + D) / 9.0
        q2v = np.ascontiguousarray(
            q2v.reshape(N_QT, 128).T.astype(np.float32)
        )

        # interpolation rows matching SEL's column order (j = 4qt+a for
        # own rows, then the gathered blocks in rank order)
        Arh = Ar[half * OROWS : (half + 1) * OROWS]      # [256, 32]
        art = np.zeros((40, OROWS), dtype=np.float32)
        for j, row in enumerate(own_rows):
            art[j] = Arh[:, row]
        for j, row in enumerate(cc_rows):
            if row not in own_rows:
                art[32 + j] = Arh[:, row]
        AcT = np.ascontiguousarray(Ac.T)                 # [32, 512]
        in_maps.append(
            {
                "dbd": dbd,
                "qd": qdc,
                "xhd": xhd,
                "q2": q2v,
                "art": art.astype(ml_dtypes.bfloat16),
                "ac4": np.tile(AcT, (4, 1)).astype(ml_dtypes.bfloat16),
                "msk4": np.repeat(
                    np.eye(4, dtype=np.float32), 32, axis=0
                ).astype(ml_dtypes.bfloat16),
            }
        )
    return in_maps


def run_device(in_maps, **kwargs):
    nc = _get_nc()
    return bass_utils.run_bass_kernel_spmd(
        nc, in_maps, core_ids=list(range(N_CORES)), **kwargs
    )


def kernel(embeddings, database, k, out_h, out_w):
    assert int(k) == K_NN and int(out_h) == OUT_H and int(out_w) == OUT_W
    in_maps = make_in_maps(np.asarray(embeddings), np.asarray(database))
    res = run_device(in_maps)
    out = np.empty((B, 1, OUT_H, OUT_W), dtype=np.float32)
    for c in range(N_CORES):
        b, half = divmod(c, 2)
        out[b, 0, half * OROWS : (half + 1) * OROWS] = res.results[c]["out"]
    return out


# revision 74
# speedup vs baseline: 1.0014x; 1.0014x over previous
"""Distributed kNN OOD-score kernel for 8 Trainium2 NeuronCores.

Problem: for each of 4*32*32 query vectors (D=768), find the 3 nearest
database vectors (N=20000, squared-L2), average the 3 distances, and
bilinearly upsample the resulting [4,32,32] map to [4,1,512,512].

Sharding: queries are data-parallel. Each core owns half of one batch
image (16 of 32 query rows = 512 queries); the database is replicated
and streamed through SBUF in fp8 (e4m3). The one halo row each core
needs for the 16x bilinear upsample is exchanged with its pair core via
a tiny AllGather whose ~15us fixed latency is hidden by hoisting the
boundary query tile (qt0) of the last two super-chunks ahead of the
other tiles (the fp8 database tiles stay resident long enough to allow
the out-of-order walk).

Per-core device program, per (super-chunk sc, query-tile qt):
  - scores t[q,n] = q.x - (||x||^2 - 768)/2 via TensorE fp8 DoubleRow
    matmuls: 3 chunks of K=256 at 0.5 cycles/row (4x bf16 rate) + one
    K=4 DoubleRow adding the centered -||x||^2/2 row in 4-way split-fp8
    precision, accumulated f32 in PSUM (8 banks, bank-major fill).
  - ScalarE evacuates PSUM banks 0-5 (two [128,3,500] copies) into a
    bf16 strip; VectorE max-fuses banks 6-7 against strip[0:1000]
    (TensorTensor may read one PSUM operand), then a bf16 pairwise-max
    ladder (2x DVE mode) reduces 4000 -> 250 candidates; per sc-pair one
    max8 keeps the top-8 (exact top-3 up to ~0.1%/query collisions in
    the depth-16 max groups, well inside tolerance).
  - final max8 over 24 strip winners -> top-3 scores; mean distance =
    reduce_sum of sqrt((q^2+768-2t)/9) (ScalarE fused scale+bias+sqrt).
  - 16x bilinear upsample = two small bf16 matmuls with interpolation
    matrices (host-built; weights are dyadic so bf16-exact).
"""

import sys

if "/opt/trn_rl_repo" not in sys.path:
    sys.path.insert(0, "/opt/trn_rl_repo")

import numpy as np
import ml_dtypes

import concourse.bass as bass
import concourse.bacc as bacc
import concourse.mybir as mybir
import concourse.tile as tile
from concourse import bass_utils

# Problem shape (hardcoded per contract).
B, D, H, W = 4, 768, 32, 32
N = 20000
K_NN = 3
OUT_H = OUT_W = 512
N_CORES = 8

SC = 4000            # db columns per super-chunk
N_SC = N // SC       # 5
BANK = 500           # db columns per PSUM bank
CH = D // 256        # 3 DoubleRow contraction chunks (K=256 each)
QPC = 512            # queries scored per core (16 rows)
N_QT = QPC // 128    # 4
OROWS = 256          # output rows per core
NCOL = 24            # ood columns entering the upsample (16 own + 2x4 gathered)

F32 = mybir.dt.float32
BF16 = mybir.dt.bfloat16
FP8 = mybir.dt.float8e4
AX = mybir.AxisListType
AF = mybir.ActivationFunctionType
DR = mybir.MatmulPerfMode.DoubleRow

# local tile -> 4-row block of this core's half (block i = rows 4i..4i+3).
# Tile 0 is the block the PAIR core needs as its halo row: for the top
# half (rows 0-15) that's block 3 (row 15), for the bottom half (rows
# 16-31) block 0 (row 16).
TILE_BLOCKS = ([3, 0, 1, 2], [0, 1, 2, 3])

# qt0-2 of the last two super-chunks are hoisted so qt0's ood (the
# boundary block) finishes ~15us before the instruction stream ends
# (hiding the AllGather's fixed latency) and only qt3's ladder + final
# remain as post-matmul tail work.
PHASES = [
    (0, (0, 1, 2, 3)),
    (1, (0, 1, 2, 3)),
    (3, (0,)),
    (4, (0,)),
    (3, (1, 2)),
    (2, (0, 1, 2, 3)),
    (4, (1, 2)),
    (3, (3,)),
    (4, (3,)),
]


def _build_program():
    nc = bacc.Bacc(
        "TRN2", target_bir_lowering=False, debug=False, num_devices=N_CORES
    )
    dbd = nc.dram_tensor("dbd", [128, N_SC, CH, 2, SC], FP8, kind="ExternalInput").ap()
    qd = nc.dram_tensor("qd", [128, N_QT, CH, 2, 128], FP8, kind="ExternalInput").ap()
    xhd = nc.dram_tensor("xhd", [2, 2, N], FP8, kind="ExternalInput").ap()
    q2 = nc.dram_tensor("q2", [128, N_QT], F32, kind="ExternalInput").ap()
    art = nc.dram_tensor("art", [40, OROWS], BF16, kind="ExternalInput").ap()
    ac4 = nc.dram_tensor("ac4", [128, OUT_W], BF16, kind="ExternalInput").ap()
    msk4 = nc.dram_tensor("msk4", [128, 4], BF16, kind="ExternalInput").ap()
    out = nc.dram_tensor("out", [OROWS, OUT_W], F32, kind="ExternalOutput").ap()

    with tile.TileContext(nc) as tc:
        with (
            tc.tile_pool(name="static", bufs=1) as sp,
            tc.tile_pool(name="db", bufs=3) as dbp,
            tc.tile_pool(name="strip", bufs=2) as stp,
            tc.tile_pool(name="lad", bufs=2) as ltp,
            tc.tile_pool(name="small", bufs=2) as smp,
            tc.tile_pool(name="psum", bufs=1, space="PSUM") as pp,
            tc.tile_pool(name="dram", bufs=1, space="DRAM") as dp,
        ):
            # DMA issue order is the DMA-engine FIFO order: the first db
            # slice leads on SP while the queries/shift rows go through the
            # Activation-issued HWDGE queue in parallel.
            qk = sp.tile([128, N_QT, CH, 2, 128], FP8, name="qk")
            nc.sync.dma_start(qk[:], qd[:])
            xh4 = sp.tile([2, 2, N], FP8, name="xh4")
            nc.sync.dma_start(xh4[:], xhd[:])
            db_t = {}
            db_t[0] = dbp.tile([128, CH, 2, SC], FP8, tag="db", name="db0")
            for h in range(4):
                nc.sync.dma_start(
                    db_t[0][:, :, :, h * 1000 : (h + 1) * 1000],
                    dbd[:, 0, :, :, h * 1000 : (h + 1) * 1000],
                )
            q2_sb = sp.tile([128, N_QT], F32, name="q2_sb")
            nc.sync.dma_start(q2_sb[:], q2[:])
            art_sb = sp.tile([40, OROWS], BF16, name="art_sb")
            nc.sync.dma_start(art_sb[:], art[:])
            ac4_sb = sp.tile([128, OUT_W], BF16, name="ac4_sb")
            nc.sync.dma_start(ac4_sb[:], ac4[:])
            msk4_sb = sp.tile([128, 4], BF16, name="msk4_sb")
            nc.sync.dma_start(msk4_sb[:], msk4[:])
            ones4 = sp.tile([2, 2, 128], FP8, name="ones4")
            nc.gpsimd.memset(ones4[:], 1.0)
            warm = sp.tile([2, 2, 512], FP8, name="warm")
            nc.gpsimd.memset(warm[:], 0.0)

            # db tiles: 4-buffer rotation; DMA-FIFO order matches the phase
            # walk (sc0, sc1, sc3, sc4/sc2 interleaved) so the hoisted qt0
            # phases never starve.  sc2 reuses sc0's buffer.
            for sc in (1, 3):
                t = dbp.tile([128, CH, 2, SC], FP8, tag="db", name=f"db{sc}")
                db_t[sc] = t
                for h in range(2):
                    nc.sync.dma_start(
                        t[:, :, :, h * 2000 : (h + 1) * 2000],
                        dbd[:, sc, :, :, h * 2000 : (h + 1) * 2000],
                    )
            db_t[4] = dbp.tile([128, CH, 2, SC], FP8, tag="db", name="db4")
            db_t[2] = dbp.tile([128, CH, 2, SC], FP8, tag="db", name="db2")
            for sc, h in ((4, 0), (4, 1), (2, 0), (2, 1)):
                nc.sync.dma_start(
                    db_t[sc][:, :, :, h * 2000 : (h + 1) * 2000],
                    dbd[:, sc, :, :, h * 2000 : (h + 1) * 2000],
                )

            # PSUM: four 2-bank pair tiles; banks 0-5 (ScalarE evac) and
            # 6-7 (DVE fused max) in the standard per-scqt walk, one pair
            # per query tile in the DMA-paced phase 0.
            P = [
                pp.tile([128, 2, 512], F32, tag=f"p{i}", name=f"P{i}")
                for i in range(4)
            ]

            # p-state warmup: ~3us of throwaway PE work during the db DMA
            # wait so the first real matmuls run at full clock
            for _ in range(6):
                nc.tensor.matmul(
                    P[3][:, 0, :], ones4[:], warm[:], start=True, stop=True,
                    perf_mode=DR,
                )

            Dbuf = sp.tile([128, N_QT, 4, 250], BF16, name="Dbuf")
            # per-qt final candidates: [0:16] = max8 of sc-pairs (0,1) and
            # (2,3); [16:516] = sc4's ladder output at the 500-wide level
            # (skips two reduction hops on the critical tail)
            parts = sp.tile([128, N_QT, 516], BF16, name="parts")
            oods_all = sp.tile([128, N_QT], F32, name="oods_all")
            ood0_bf = sp.tile([128, 1], BF16, name="ood0_bf")
            # SEL[p, 4qt+a] = ood[p, qt] * [p//32 == a]; SELc holds the
            # gathered boundary blocks on partitions 0-31 (zero elsewhere).
            # With ac4[p, ow] = Ac.T[p%32, ow] the K=128 matmuls SEL.T @ ac4
            # implicitly transpose lanes.  Separate tiles so the AllGather's
            # DMA into SELc carries no false dep on the qt3 SEL write.
            SEL = sp.tile([128, 16], BF16, name="SEL")
            SELc = sp.tile([128, 8], BF16, name="SELc")
            nc.gpsimd.memset(SELc[:], 0.0)
            cc_in = dp.tile([128], BF16, name="cc_in")
            cc_out = dp.tile([256], BF16, name="cc_out")

            def emit_mms(pt, qt, sc, col):
                o = pt[:, (col // BANK) % 2, 0:BANK]
                for ch in range(CH):
                    nc.tensor.matmul(
                        o,
                        qk[:, qt, ch],
                        db_t[sc][:, ch, :, col : col + BANK],
                        start=(ch == 0),
                        stop=False,
                        perf_mode=DR,
                    )
                nc.tensor.matmul(
                    o,
                    ones4[:],
                    xh4[:, :, sc * SC + col : sc * SC + col + BANK],
                    start=False,
                    stop=True,
                    perf_mode=DR,
                )

            def emit_scqt(sc, qt):
                # bank-major fill so PSUM groups complete in evac order
                for b in range(8):
                    emit_mms(P[b // 2], qt, sc, BANK * b)
                strip = stp.tile([128, 3000], BF16, tag="strip", name="strip")
                sv = strip.rearrange("p (a b n) -> p a b n", a=3, b=2)
                for i in range(3):
                    nc.scalar.activation(sv[:, i], P[i][:, :, 0:BANK], AF.Copy)
                # pairwise-max ladder: 4000 -> 250 exact-value candidates
                Ht = ltp.tile([128, 1000], BF16, tag="H", name="Ht")
                nc.vector.tensor_max(Ht[:], P[3][:, :, 0:BANK], strip[:, 0:1000])
                At = ltp.tile([128, 1000], BF16, tag="A", name="At")
                nc.vector.tensor_max(At[:], strip[:, 1000:2000], strip[:, 2000:3000])
                Bt = ltp.tile([128, 1000], BF16, tag="Bv", name="Bt")
                nc.vector.tensor_max(Bt[:], At[:], Ht[:])
                if sc < 4:
                    Ct = ltp.tile([128, 500], BF16, tag="C", name="Ct")
                    nc.vector.tensor_max(Ct[:], Bt[:, 0:500], Bt[:, 500:1000])
                    nc.vector.tensor_max(
                        Dbuf[:, qt, sc], Ct[:, 0:250], Ct[:, 250:500]
                    )
                else:
                    nc.vector.tensor_max(
                        parts[:, qt, 16:516], Bt[:, 0:500], Bt[:, 500:1000]
                    )

            # Phase 0 is paced by the sc0 DMA stream: process all four
            # query tiles per arriving 1000-col quarter (one PSUM pair
            # each).  qt0-2 evacuate via ScalarE into per-qt strips; qt3's
            # quarters fold into a running pairwise max on DVE.
            ph0s = sp.tile([128, 3, SC], BF16, name="ph0s")
            h3 = [
                sp.tile([128, 1000], BF16, name=f"h3{i}") for i in range(2)
            ]
            ph0sv = ph0s.rearrange("p q (r b n) -> p q r b n", r=4, b=2)

            def emit_phase0():
                for r in range(4):
                    for qt in range(N_QT):
                        for b2 in range(2):
                            emit_mms(P[qt], qt, 0, 1000 * r + BANK * b2)
                    for qt in range(3):
                        nc.scalar.activation(
                            ph0sv[:, qt, r], P[qt][:, :, 0:BANK], AF.Copy
                        )
                    if r == 0:
                        nc.vector.tensor_copy(h3[0][:], P[3][:, :, 0:BANK])
                    else:
                        nc.vector.tensor_max(
                            h3[r % 2][:], P[3][:, :, 0:BANK], h3[1 - r % 2][:]
                        )
                for qt in range(3):
                    s = ph0s[:, qt]
                    A = ltp.tile([128, 2000], BF16, tag="H", name="A0")
                    nc.vector.tensor_max(A[:], s[:, 0:2000], s[:, 2000:4000])
                    Bt = ltp.tile([128, 1000], BF16, tag="A", name="B0")
                    nc.vector.tensor_max(Bt[:], A[:, 0:1000], A[:, 1000:2000])
                    Ct = ltp.tile([128, 500], BF16, tag="C", name="C0")
                    nc.vector.tensor_max(Ct[:], Bt[:, 0:500], Bt[:, 500:1000])
                    nc.vector.tensor_max(
                        Dbuf[:, qt, 0], Ct[:, 0:250], Ct[:, 250:500]
                    )
                C3 = ltp.tile([128, 500], BF16, tag="C", name="C3")
                nc.vector.tensor_max(C3[:], h3[1][:, 0:500], h3[1][:, 500:1000])
                nc.vector.tensor_max(Dbuf[:, 3, 0], C3[:, 0:250], C3[:, 250:500])

            def emit_pair_max8(qt, pair_idx):
                i0 = pair_idx * 8
                src = Dbuf[:, qt, 2 * pair_idx : 2 * pair_idx + 2]
                nc.vector.max(parts[:, qt, i0 : i0 + 8], src)

            def emit_qt_final(qt):
                f8 = smp.tile([128, 8], BF16, tag="f8", name="f8")
                nc.vector.max(f8[:], parts[:, qt])
                # dist_j/3 = sqrt((q2 + 768 - 2 t_j) / 9); host passes
                # (q2+768)/9 as the bias
                d3 = smp.tile([128, K_NN], BF16, tag="d3", name="d3")
                nc.scalar.activation(
                    d3[:],
                    f8[:, 0:K_NN],
                    AF.Sqrt,
                    bias=q2_sb[:, qt : qt + 1],
                    scale=-2.0 / 9.0,
                    accum_out=oods_all[:, qt : qt + 1],
                )
                nc.vector.tensor_scalar_mul(
                    SEL[:, 4 * qt : 4 * qt + 4],
                    msk4_sb[:],
                    oods_all[:, qt : qt + 1],
                )
                if qt == 0:
                    # boundary block: gather across the pair while qt1-3
                    # of sc3/sc4 still run (~25us of cover); issue from the
                    # otherwise-idle Pool queue so it never waits on SP
                    nc.vector.tensor_copy(ood0_bf[:], oods_all[:, 0:1])
                    nc.gpsimd.dma_start(cc_in[:], ood0_bf[:])
                    nc.gpsimd.collective_compute(
                        "AllGather",
                        mybir.AluOpType.bypass,
                        replica_groups=[[0, 1], [2, 3], [4, 5], [6, 7]],
                        ins=[cc_in.opt()],
                        outs=[cc_out.opt()],
                    )
                    nc.sync.dma_start(
                        SELc[0:32, :],
                        cc_out.rearrange("(b r c) -> c (b r)", b=2, c=W),
                    )

            # qt0 walks its super-chunks in order 0,1,3,4,2 so its final
            # (and the AllGather) launches ~25us before the stream ends;
            # qt1/qt2 walk 0,1,3,2,4 and qt3 walks in order, so the phase
            # sequence never outruns the db DMA stream.
            for sc, qts in PHASES:
                if sc == 0:
                    emit_phase0()
                    continue
                for qt in qts:
                    emit_scqt(sc, qt)
                    if sc == 1:
                        emit_pair_max8(qt, 0)
                    elif sc == 2 and qt == 0:
                        emit_pair_max8(0, 1)
                        emit_qt_final(0)
                    elif sc == 2 and qt in (1, 2):
                        emit_pair_max8(qt, 1)
                    elif sc == 3 and qt == 3:
                        emit_pair_max8(3, 1)
                    elif sc == 4 and qt != 0:
                        emit_qt_final(qt)

            # Upsample: out = art.T @ (SEL.T @ ac4); the K=128 first stage
            # transposes the ood lanes implicitly.  p1p rows 0:16 = own, 32:40
            # = gathered (PE column offset 32 is the only legal slot for an
            # M=8 output); rows 16:32 are stale-but-finite PSUM ignored via
            # zero rows of art.
            p1p = pp.tile([40, OUT_W], F32, tag="p0", name="p1p")
            nc.tensor.matmul(
                p1p[32:40, :], SELc[:], ac4_sb[:], start=True, stop=True,
                tile_position=(0, 32),
            )
            nc.tensor.matmul(p1p[0:16, :], SEL[:], ac4_sb[:], start=True, stop=True)
            p1_sb = sp.tile([40, OUT_W], BF16, name="p1_sb")
            nc.scalar.activation(p1_sb[:], p1p[:], AF.Copy)
            # parallel output ceremony: half 0 via Act copy + SP DMA,
            # half 1 via DVE copy + Act-issued DMA
            for m in range(2):
                p2p = pp.tile(
                    [128, OUT_W], F32, tag=("p1", "p2")[m], name=f"p2p{m}"
                )
                nc.tensor.matmul(
                    p2p[:],
                    art_sb[:, m * 128 : (m + 1) * 128],
                    p1_sb[:],
                    start=True,
                    stop=True,
                )
                o_sb = smp.tile([128, OUT_W], F32, tag="osb", name=f"osb{m}")
                if m == 0:
                    nc.scalar.activation(o_sb[:], p2p[:], AF.Copy)
                    nc.sync.dma_start(out[0:128, :], o_sb[:])
                else:
                    nc.vector.tensor_copy(o_sb[:], p2p[:])
                    nc.scalar.dma_start(out[128:256, :], o_sb[:])

    nc.compile()
    return nc


def _bilinear_matrix(out_size: int, in_size: int) -> np.ndarray:
    """Half-pixel (align_corners=False) bilinear interpolation matrix
    [out_size, in_size]; edge-clamped, equivalent to jax.image.resize
    'bilinear' for integer upsampling."""
    A = np.zeros((out_size, in_size), dtype=np.float64)
    scale = in_size / out_size
    for i in range(out_size):
        s = (i + 0.5) * scale - 0.5
        j0 = int(np.floor(s))
        w = s - j0
        A[i, min(max(j0, 0), in_size - 1)] += 1.0 - w
        A[i, min(max(j0 + 1, 0), in_size - 1)] += w
    return A.astype(np.float32)


_NC_CACHE = None


def _get_nc():
    global _NC_CACHE
    if _NC_CACHE is None:
        _NC_CACHE = _build_program()
    return _NC_CACHE


def _fp8_split4(v: np.ndarray) -> np.ndarray:
    """4-way residual split of f32 vector v into fp8 e4m3 rows that sum
    (in f32) back to v to ~1e-4 absolute."""
    rows = []
    r = v.astype(np.float64)
    for _ in range(4):
        s = np.asarray(r, dtype=np.float32).astype(ml_dtypes.float8_e4m3)
        rows.append(s)
        r = r - s.astype(np.float64)
    return np.stack(rows)  # [4, N]


def make_in_maps(embeddings: np.ndarray, database: np.ndarray):
    embeddings = np.asarray(embeddings, dtype=np.float32)
    database = np.asarray(database, dtype=np.float32)

    x8 = database.astype(ml_dtypes.float8_e4m3)          # [N, D]
    # dbd[p, sc, ch, i, n] = x8[sc*SC+n, 256ch+128i+p]
    dbT = np.ascontiguousarray(x8.T)                     # [D, N]
    dbd = np.ascontiguousarray(
        dbT.reshape(CH, 2, 128, N_SC, SC).transpose(2, 3, 0, 1, 4)
    )
    # centered score shift s = (768 - ||x8||^2)/2, 4-way split fp8
    xs = x8.astype(np.float32)
    s = (D - np.einsum("nd,nd->n", xs, xs)) * 0.5
    sp4 = _fp8_split4(s)                                 # [4, N]
    xhd = np.ascontiguousarray(sp4.reshape(2, 2, N).transpose(1, 0, 2))

    q_all = embeddings.transpose(0, 2, 3, 1).reshape(B, H * W, D)
    Ac = _bilinear_matrix(OUT_W, W)                      # [512, 32]
    Ar = _bilinear_matrix(OUT_H, H)                      # [512, 32]
    # the two gathered blocks in cc_out rank order: pair-core tile 0 rows
    cc_rows = [12, 13, 14, 15, 16, 17, 18, 19]

    in_maps = []
    for c in range(N_CORES):
        b, half = divmod(c, 2)
        blocks = TILE_BLOCKS[half]
        own_rows = [16 * half + 4 * blk + r for blk in blocks for r in range(4)]

        # queries in local-tile order
        q = np.concatenate(
            [
                q_all[b, (16 * half + 4 * blk) * W : (16 * half + 4 * blk + 4) * W]
                for blk in blocks
            ]
        )                                                # [512, 768]
        q8 = q.astype(ml_dtypes.float8_e4m3)
        # qd[p, qt, ch, i, m] = q8[128qt+m, 256ch+128i+p]
        qT = np.ascontiguousarray(q8.T)                  # [768, 512]
        qdc = np.ascontiguousarray(
            qT.reshape(CH, 2, 128, N_QT, 128).transpose(2, 3, 0, 1, 4)
        )
        q8f = q8.astype(np.float32)
        q2v = (np.einsum("qd,qd->q", q8f, q8f) + D) / 9.0
        q2v = np.ascontiguousarray(
            q2v.reshape(N_QT, 128).T.astype(np.float32)
        )

        # interpolation rows matching SEL's column order (j = 4qt+a for
        # own rows, then the gathered blocks in rank order)
        Arh = Ar[half * OROWS : (half + 1) * OROWS]      # [256, 32]
        art = np.zeros((40, OROWS), dtype=np.float32)
        for j, row in enumerate(own_rows):
            art[j] = Arh[:, row]
        for j, row in enumerate(cc_rows):
            if row not in own_rows:
                art[32 + j] = Arh[:, row]
        AcT = np.ascontiguousarray(Ac.T)                 # [32, 512]
        in_maps.append(
            {
                "dbd": dbd,
                "qd": qdc,
                "xhd": xhd,
                "q2": q2v,
                "art": art.astype(ml_dtypes.bfloat16),
                "ac4": np.tile(AcT, (4, 1)).astype(ml_dtypes.bfloat16),
                "msk4": np.repeat(
                    np.eye(4, dtype=np.float32), 32, axis=0
                ).astype(ml_dtypes.bfloat16),
            }
        )
    return in_maps


def run_device(in_maps, **kwargs):
    nc = _get_nc()
    return bass_utils.run_bass_kernel_spmd(
        nc, in_maps, core_ids=list(range(N_CORES)), **kwargs
    )


def kernel(embeddings, database, k, out_h, out_w):
    assert int(k) == K_NN and int(out_h) == OUT_H and int(out_w) == OUT_W
    in_maps = make_in_maps(np.asarray(embeddings), np.asarray(database))
    res = run_device(in_maps)
    out = np.empty((B, 1, OUT_H, OUT_W), dtype=np.float32)
    for c in range(N_CORES):
        b, half = divmod(c, 2)
        out[b, 0, half * OROWS : (half + 1) * OROWS] = res.results[c]["out"]
    return out
